# revision 41
# baseline (speedup 1.0000x reference)
"""Trainium2 Bass kernel for nn_AttentionBlock (B=32, C=256, H*W=1024 tokens,
4 heads x 64 dim, out-proj + residual).

Sharding: data-parallel over batch -- 8 cores x 4 batches each.

Per-core dataflow (everything stays in transposed [feature, token] layout, so
no tensor transposes are needed anywhere):

  x[b]            : [C=256, S=1024]  (natural layout of the input!)
  qkT[hp,t]       : [128, S] head-PAIR tiles (head 2hp rows 0-63, head 2hp+1
                    rows 64-127; t=0 -> q, t=1 -> k) = W_slice.T @ x[b]
  v_aug[t]        : [128 tok, 4*(64 v + 1 ones)]   = x_chunk.T @ W_v  (+bias
                    via a precomputed broadcast tile in the DVE drain)
  scores^T (psum) : [j, i] = kT.T @ q_pad -- full-K=128 matmuls where the
                    zero-padded half of the per-head q operand annihilates the
                    other head's k rows.  EVERY matmul in the kernel runs
                    K=128 at base partition 0: mixing PE tiling modes drains
                    the array between matmuls (measured 722 ns/MM mixed vs
                    ~227 ns uniform), so K=64/K=1 shapes are avoided entirely.
  E = exp(scores^T * 0.125)  (ACT, psum->sbuf, FD=1024 per op; the
                    (N+352)-cycle ACTIVATE cost is the throughput floor)
  PV (psum)       : [65, i] rows 0-63 = v_h.T @ E = attnout^T (unnormalized),
                    row 64 = column sums of E = softmax denominators
                    (the ones-column of v_aug rides along for free)
  normalize       : denominator row DMA-spread to [32,16] -> DVE reciprocal on
                    32 lanes -> DMA back into a persistent zero-padded row ->
                    K=128 broadcast matmul against a ones-row-0 stationary ->
                    DVE multiply -> attnT[hp] [128 (2 heads), S]
  y^T             : [C, S] = W_out.T @ attnT + b_out + x[b]  -> output, which
                    is already the required [B, C, H, W] layout.

Matmul dtype: float32r (single-pass PE fp32, ~1.6e-4 matmul rel err measured
on HW vs 2.4e-3 for bf16).  All tiles feeding matmuls are produced as f32r;
the residual add stays exact f32.

Emission is software-pipelined at instruction granularity: each head-pair
unit's merged loop carries this unit's scores+exp, the PREVIOUS unit's PV
matmuls, and up to two queued projection/out-projection psum groups per
iteration, so the in-order PE queue always has independent work while ACT
(the exp engine, the throughput floor) drains score tiles.
"""

import os
import numpy as np

B_FULL = 32
N_CORES = 8
B_LOC = B_FULL // N_CORES  # 4 batches per core
C = 256
S = 1024
H = 4
D = 64
SCALE = D ** -0.5  # 0.125
P = 128
NKC = C // P  # 2 contraction chunks
NI = S // 512  # 2 i-chunks of 512
NJ = S // P  # 8 j-chunks of 128

MM_MODE = os.environ.get("ATT_MM_MODE", "f32r")

_NC_CACHE = {}


def build_nc():
    import concourse.mybir as mybir
    import concourse.tile as tile
    from concourse import bacc
    from contextlib import ExitStack

    f32 = mybir.dt.float32
    edt = mybir.dt.bfloat16 if MM_MODE != "f32" else mybir.dt.float32
    mdt = {
        "f32": mybir.dt.float32,
        "f32r": mybir.dt.float32r,
        "bf16": mybir.dt.bfloat16,
    }[MM_MODE]
    Exp = mybir.ActivationFunctionType.Exp

    nc = bacc.Bacc("TRN2")

    x_d = nc.dram_tensor("x", [B_LOC, C, S], f32, kind="ExternalInput")
    wqkv_d = nc.dram_tensor("W_qkv", [C, 3 * H * D], f32, kind="ExternalInput")
    bqkv_d = nc.dram_tensor("b_qkv", [3 * H * D], f32, kind="ExternalInput")
    wout_d = nc.dram_tensor("W_out", [C, C], f32, kind="ExternalInput")
    bout_d = nc.dram_tensor("b_out", [C], f32, kind="ExternalInput")
    out_d = nc.dram_tensor("out", [B_LOC, C, S], f32, kind="ExternalOutput")

    with ExitStack() as ctx:
        ctx.enter_context(
            nc.allow_low_precision(reason="f32r/bf16 matmul feed tiles by design")
        )
        tc = ctx.enter_context(tile.TileContext(nc))
        const = ctx.enter_context(tc.tile_pool(name="const", bufs=1))

        # ---- constants: DMA f32 staging, cast to matmul dtype ----
        with tc.tile_pool(name="staging", bufs=1) as stg:
            wqk_f = stg.tile([P, NKC, 4, P], f32)
            wv_f = stg.tile([P, NKC, H * D], f32)
            wout_f = stg.tile([P, NKC, C], f32)
            bv_f = stg.tile([1, H * D], f32)
            wq_dram = wqkv_d.rearrange(
                "(kc p) (h t d) -> p kc h t d", p=P, h=H, t=3
            )
            for kc in range(NKC):
                for hp in range(2):
                    for t in range(2):
                        nc.gpsimd.dma_start(
                            wqk_f[:, kc, hp * 2 + t, :].rearrange(
                                "p (a d) -> p a d", a=2
                            ),
                            wq_dram[:, kc, 2 * hp : 2 * hp + 2, t, :],
                        )
                nc.gpsimd.dma_start(
                    wv_f[:, kc, :].rearrange("p (h d) -> p h d", h=H),
                    wq_dram[:, kc, :, 2, :],
                )
            nc.gpsimd.dma_start(wout_f, wout_d.rearrange("(kc p) n -> p kc n", p=P))
            nc.gpsimd.dma_start(
                bv_f, bqkv_d.rearrange("(h t d) -> h t d", h=H, t=3)[None, :, 2, :]
            )

            wqk_sb = const.tile([P, NKC, 4, P], mdt)
            wv_sb = const.tile([P, NKC, H * D], mdt)
            wout_sb = const.tile([P, NKC, C], mdt)
            bv_row = const.tile([1, H * D], mdt)
            if mdt == f32:
                wqk_sb, wv_sb, wout_sb, bv_row = wqk_f, wv_f, wout_f, bv_f
            else:
                nc.vector.tensor_copy(out=wqk_sb, in_=wqk_f)
                nc.vector.tensor_copy(out=wv_sb, in_=wv_f)
                nc.vector.tensor_copy(out=wout_sb, in_=wout_f)
                nc.vector.tensor_copy(out=bv_row, in_=bv_f)

        bqk_sb = const.tile([P, 2, 2], f32)
        bq4 = bqkv_d.rearrange("(hp a t d) -> hp a t d", hp=2, a=2, t=3)
        for hp in range(2):
            for t in range(2):
                for a in range(2):
                    nc.gpsimd.dma_start(
                        bqk_sb[64 * a : 64 * a + 64, hp, t : t + 1],
                        bq4[hp, a, t, :, None],
                    )
        bout_sb = const.tile([P, NKC], f32)
        nc.gpsimd.dma_start(bout_sb, bout_d.rearrange("(mc p) -> p mc", p=P))

        def memset_one(ap):
            if MM_MODE == "f32r":
                nc.vector.memset(ap.bitcast(mybir.dt.uint32), 0x3F800000)
            else:
                nc.vector.memset(ap, 1.0)

        # ones_row0: [128, 128] with row 0 = 1, rows 1-127 = 0.  Used as the
        # K=128 stationary operand of "broadcast row 0 of rhs to M partitions"
        # matmuls (K=1 matmuls would flip the PE into a different tiling mode).
        ones_row0 = const.tile([P, P], mdt)
        nc.vector.memset(ones_row0.bitcast(mybir.dt.uint32), 0)
        memset_one(ones_row0[0:1, :])
        # persistent zero-padded rows for the reciprocal-broadcast rhs: row 0
        # gets DMA'd per use, rows 1-127 stay zero forever (finite, so the
        # zero weights of ones_row0 annihilate them)
        rcr_slots = []
        for i_ in range(3):
            t_ = const.tile([P, 512], mdt, name=f"rcr_slot{i_}", tag=f"rcrs{i_}")
            nc.vector.memset(t_.bitcast(mybir.dt.uint32), 0)
            rcr_slots.append(t_)
        # broadcast the v-bias row to all 128 partitions once, so the
        # per-tile v bias becomes a plain DVE add
        bvp = const.tile([P, H * D], mdt)
        nc.vector.memset(bvp.bitcast(mybir.dt.uint32), 0)
        nc.vector.tensor_copy(out=bvp[0:1, :], in_=bv_row)
        with tc.tile_pool(name="ps_init", bufs=1, space="PSUM") as ps_init:
            pbv = ps_init.tile([P, H * D], f32)
            nc.tensor.matmul(pbv, lhsT=ones_row0, rhs=bvp, start=True, stop=True)
            bv_bcast = const.tile([P, H * D], f32)
            nc.vector.tensor_copy(out=bv_bcast, in_=pbv)

        xp = ctx.enter_context(tc.tile_pool(name="xp", bufs=3))
        xrp = ctx.enter_context(tc.tile_pool(name="xrp", bufs=1))
        qkp = ctx.enter_context(tc.tile_pool(name="qkp", bufs=12))
        vp = ctx.enter_context(tc.tile_pool(name="vp", bufs=16))
        ep = ctx.enter_context(tc.tile_pool(name="ep", bufs=24))
        ap_ = ctx.enter_context(tc.tile_pool(name="ap", bufs=4))
        ahp = ctx.enter_context(tc.tile_pool(name="ahp", bufs=1))
        unp = ctx.enter_context(tc.tile_pool(name="unp", bufs=5))
        dspp = ctx.enter_context(tc.tile_pool(name="dspp", bufs=3))
        rrp = ctx.enter_context(tc.tile_pool(name="rrp", bufs=3))
        rcp = ctx.enter_context(tc.tile_pool(name="rcp", bufs=3))
        yp = ctx.enter_context(tc.tile_pool(name="yp", bufs=1))
        ps_s = ctx.enter_context(tc.tile_pool(name="ps_s", bufs=2, space="PSUM"))
        ps_pv = ctx.enter_context(tc.tile_pool(name="ps_pv", bufs=2, space="PSUM"))
        ps_p = ctx.enter_context(tc.tile_pool(name="ps_p", bufs=1, space="PSUM"))
        ps_b = ctx.enter_context(tc.tile_pool(name="ps_b", bufs=1, space="PSUM"))

        def mm64(out, lhsT, rhs, start, stop):
            # plain full-array matmul; every matmul in this kernel runs with
            # K=128 at base partition 0 so the PE tiling mode NEVER changes
            # (mode switches drain the array: measured 722 ns/MM in
            # mixed-mode streams vs ~227 in a uniform full-K stream)
            nc.tensor.matmul(out, lhsT=lhsT, rhs=rhs, start=start, stop=stop)

        x_sb = {}
        x_r = {}
        qkT = {}
        v_aug = {}
        attnT = {}

        # queue of deferred psum-group emitters (proj/outproj), consumed <=2
        # per merged-loop iteration so the in-order PE never head-of-line
        # blocks on a DVE psum drain
        aux_q = []

        def drain_aux(n):
            for _ in range(min(n, len(aux_q))):
                aux_q.pop(0)()

        def load_x(b):
            t = xp.tile([P, NKC, S], f32, name=f"x_{b}", tag="x")
            nc.sync.dma_start(t, x_d[b].rearrange("(kc p) s -> p kc s", p=P))
            x_sb[b] = t

        def queue_proj(b):
            xt = x_sb[b]
            if mdt == f32:
                x_r[b] = xt
            else:
                xr = xrp.tile([P, NKC, S], mdt, name=f"xr_{b}", tag="xr")
                nc.vector.tensor_copy(out=xr, in_=xt)
                x_r[b] = xr
            qkT[b] = {}
            v_aug[b] = [None] * NJ
            qpad = {}
            for hp in range(2):
                qkT[b][(hp, 1)] = qkp.tile(
                    [P, S], mdt, name=f"kT_{b}_{hp}", tag="qkT"
                )
                for a in range(2):
                    h = 2 * hp + a
                    qp = qkp.tile([P, S], mdt, name=f"qpad_{b}_{h}", tag="qkT")
                    # zero the other head's half once; the zero rows make the
                    # full-K=128 scores matmul select only this head
                    nc.gpsimd.memset(
                        qp[64 * (1 - a) : 64 * (1 - a) + 64, :].bitcast(
                            mybir.dt.uint32
                        ),
                        0,
                    )
                    qpad[h] = qp
            qkT[b]["qpad"] = qpad

            def qk_group(b, hp, t, n):
                def emit():
                    pq = ps_p.tile([P, 512], f32, name="pq", tag="ps_p")
                    for kc in range(NKC):
                        mm64(
                            pq,
                            wqk_sb[:, kc, hp * 2 + t, :],
                            x_r[b][:, kc, 512 * n : 512 * (n + 1)],
                            start=(kc == 0),
                            stop=(kc == NKC - 1),
                        )
                    if t == 1:
                        nc.vector.tensor_scalar_add(
                            qkT[b][(hp, 1)][:, 512 * n : 512 * (n + 1)],
                            pq,
                            bqk_sb[:, hp, t : t + 1],
                        )
                    else:
                        for a in range(2):
                            h = 2 * hp + a
                            nc.vector.tensor_scalar_add(
                                qkT[b]["qpad"][h][
                                    64 * a : 64 * a + 64,
                                    512 * n : 512 * (n + 1),
                                ],
                                pq[64 * a : 64 * a + 64],
                                bqk_sb[64 * a : 64 * a + 64, hp, t : t + 1],
                            )

                return emit

            def v_group(b, t):
                def emit():
                    pv = ps_p.tile([P, 512], f32, name="pv", tag="ps_p")
                    pvv = pv[:, 0 : H * D]
                    for kc in range(NKC):
                        mm64(
                            pvv,
                            x_r[b][:, kc, P * t : P * (t + 1)],
                            wv_sb[:, kc, :],
                            start=(kc == 0),
                            stop=(kc == NKC - 1),
                        )
                    vt = vp.tile(
                        [P, H * (D + 1)], edt, name=f"vaug_{b}_{t}", tag="vaug"
                    )
                    nc.vector.tensor_add(
                        out=vt.rearrange("p (h e) -> p h e", h=H)[:, :, 0:D],
                        in0=pvv.rearrange("p (h d) -> p h d", h=H),
                        in1=bv_bcast.rearrange("p (h d) -> p h d", h=H),
                    )
                    nc.vector.memset(
                        vt.rearrange("p (h e) -> p h e", h=H)[:, :, D : D + 1], 1.0
                    )
                    v_aug[b][t] = vt

                return emit

            for hp in range(2):
                for t in range(2):
                    for n in range(NI):
                        aux_q.append(qk_group(b, hp, t, n))
            for t in range(NJ):
                aux_q.append(v_group(b, t))

        def queue_outproj(b):
            yt = yp.tile([P, NKC, S], f32, name=f"y_{b}", tag="y")

            def out_group(mc, ic, last):
                def emit():
                    py = ps_p.tile([P, 512], f32, name="py", tag="ps_p")
                    for kc in range(NKC):
                        mm64(
                            py,
                            wout_sb[:, kc, P * mc : P * (mc + 1)],
                            attnT[b][kc][:, 512 * ic : 512 * (ic + 1)],
                            start=(kc == 0),
                            stop=(kc == NKC - 1),
                        )
                    nc.vector.scalar_tensor_tensor(
                        out=yt[:, mc, 512 * ic : 512 * (ic + 1)],
                        in0=py,
                        scalar=bout_sb[:, mc : mc + 1],
                        in1=x_sb[b][:, mc, 512 * ic : 512 * (ic + 1)],
                        op0=mybir.AluOpType.add,
                        op1=mybir.AluOpType.add,
                    )
                    if last:
                        nc.sync.dma_start(
                            out_d[b].rearrange("(kc p) s -> p kc s", p=P), yt
                        )

                return emit

            # out-projection groups release the oldest x tile; they must
            # drain BEFORE queued projections of future batches, whose x load
            # is waiting for that very slot (else: scheduling deadlock)
            groups = [
                out_group(mc, ic, mc == NKC - 1 and ic == NI - 1)
                for mc in range(NKC)
                for ic in range(NI)
            ]
            aux_q[0:0] = groups

        norm_count = [0]

        def norm_chain(b, hp, a, ic, po, dst):
            """softmax-normalize po rows 0-63 into dst[:, ic*512:...]."""
            un = unp.tile([65, 512], f32, name="un", tag="un")
            nc.vector.tensor_copy(out=un, in_=po)  # frees po early (rows+den)
            dsp = dspp.tile([32, 16], f32, name="dsp", tag="dsp")
            nc.gpsimd.dma_start(dsp, un[64:65, 0:512])  # spread row over 32 lanes
            rr = rrp.tile([32, 16], mdt, name="rr", tag="rr")
            nc.vector.reciprocal(out=rr, in_=dsp)
            rcr = rcr_slots[norm_count[0] % len(rcr_slots)]
            norm_count[0] += 1
            nc.gpsimd.dma_start(rcr[0:1, 0:512], rr)
            pb = ps_b.tile([P, 512], f32, name="pb", tag="ps_b")
            nc.tensor.matmul(
                pb[0:64], lhsT=ones_row0[:, 0:64], rhs=rcr, start=True, stop=True
            )
            nc.vector.tensor_mul(
                out=dst[:, 512 * ic : 512 * (ic + 1)], in0=un[0:64], in1=pb[0:64]
            )

        def unit(u, prev):
            """Emit head-pair unit u = (b, hp): scores+exp for its two heads,
            interleaved with PV+normalize of the previous unit and queued
            projection groups."""
            b, hp = u
            qp0 = qkT[b]["qpad"][2 * hp]
            qp1 = qkT[b]["qpad"][2 * hp + 1]
            k = qkT[b][(hp, 1)]
            E0 = [None] * NJ
            E1 = [None] * NJ
            if prev is not None:
                pb_, php_, pE0, pE1 = prev
                if php_ == 0 and pb_ not in attnT:
                    attnT[pb_] = [
                        ap_.tile([P, S], mdt, name=f"attnT_{pb_}_{kk}", tag="attnT")
                        for kk in range(NKC)
                    ]
                po = {}  # (a, ic) -> psum accumulator (allocated lazily)
                ah = {}
            for jc in range(NJ):
                # consume queued psum-groups BEFORE this iteration's scores so
                # their matmuls sit ahead in the in-order PE queue (unit (0,0)
                # drains 4/iter to fill its own projection inputs in time)
                drain_aux(4 if (b, hp) == (0, 0) else 2)
                ps0 = ps_s.tile([P, 1024], f32, name="ps0", tag="ps2")
                ps1 = ps_s.tile([P, 1024], f32, name="ps1", tag="ps2")
                E0[jc] = ep.tile([P, S], edt, name=f"E0_{b}_{hp}_{jc}", tag="E")
                E1[jc] = ep.tile([P, S], edt, name=f"E1_{b}_{hp}_{jc}", tag="E")
                for a, qp, psx in ((0, qp0, ps0), (1, qp1, ps1)):
                    for ic in range(NI):
                        # per-head scores as full-K=128 matmuls: lhsT is the
                        # combined k tile (both heads); the zero rows of the
                        # padded q operand select the wanted head
                        nc.tensor.matmul(
                            psx[:, 512 * ic : 512 * (ic + 1)],
                            lhsT=k[:, P * jc : P * (jc + 1)],
                            rhs=qp[:, 512 * ic : 512 * (ic + 1)],
                            start=True,
                            stop=True,
                        )
                nc.scalar.activation(out=E0[jc], in_=ps0, func=Exp, scale=SCALE)
                nc.scalar.activation(out=E1[jc], in_=ps1, func=Exp, scale=SCALE)
                if prev is not None:
                    # PV of prev unit: ic0 during iters 0-3, ic1 during 4-7
                    icp = jc // 4
                    for sub in range(2):
                        jj = (jc % 4) * 2 + sub
                        for a in range(2):
                            h = 2 * php_ + a
                            if jj == 0:
                                po[(a, icp)] = ps_pv.tile(
                                    [65, 512], f32, name="po", tag="po"
                                )
                            pE = pE0 if a == 0 else pE1
                            mm64(
                                po[(a, icp)],
                                v_aug[pb_][jj][:, 65 * h : 65 * h + 65],
                                pE[jj][:, 512 * icp : 512 * (icp + 1)],
                                start=(jj == 0),
                                stop=(jj == NJ - 1),
                            )
                    if jc % 4 == 3:
                        # this icp pair just stopped -> normalize now
                        for a in range(2):
                            if a == 0:
                                dst = attnT[pb_][php_][0:64]
                            else:
                                if a not in ah:
                                    ah[a] = ahp.tile(
                                        [64, S], mdt, name=f"ah_{pb_}_{php_}", tag="ah"
                                    )
                                dst = ah[a]
                            norm_chain(pb_, php_, a, icp, po[(a, icp)], dst)
            if prev is not None and 1 in ah:
                nc.gpsimd.dma_start(attnT[pb_][php_][64:128, :], ah[1])
            return (b, hp, E0, E1)

        # ---- pipeline over head-pair units ----
        units = [(b, hp) for b in range(B_LOC) for hp in range(2)]
        load_x(0)
        queue_proj(0)  # drained inside unit (0,0)'s iterations
        if B_LOC > 1:
            load_x(1)
        prev = None
        for b, hp in units:
            if hp == 0 and b + 1 < B_LOC:
                queue_proj(b + 1)
                if b + 2 < B_LOC:
                    load_x(b + 2)
            prev_done = prev
            prev = unit((b, hp), prev)
            if prev_done is not None and prev_done[1] == 1:
                queue_outproj(prev_done[0])
        # drain: PV + norms of the last unit, then remaining aux work
        b, hp, E0, E1 = prev
        if hp == 1:
            pass
        po = {}
        if b not in attnT:
            attnT[b] = [
                ap_.tile([P, S], mdt, name=f"attnT_{b}_{kk}", tag="attnT")
                for kk in range(NKC)
            ]
        ah_last = None
        for ic in range(NI):
            for a in range(2):
                h = 2 * hp + a
                po[(a, ic)] = ps_pv.tile([65, 512], f32, name="po", tag="po")
                E = E0 if a == 0 else E1
                for jj in range(NJ):
                    mm64(
                        po[(a, ic)],
                        v_aug[b][jj][:, 65 * h : 65 * h + 65],
                        E[jj][:, 512 * ic : 512 * (ic + 1)],
                        start=(jj == 0),
                        stop=(jj == NJ - 1),
                    )
            # both heads' accumulators done -> emit their chains together so
            # the two spread/gather DMA latencies overlap
            for a in range(2):
                if a == 0:
                    dst = attnT[b][hp][0:64]
                else:
                    if ah_last is None:
                        ah_last = ahp.tile([64, S], mdt, name="ah_last", tag="ah")
                    dst = ah_last
                norm_chain(b, hp, a, ic, po[(a, ic)], dst)
        nc.gpsimd.dma_start(attnT[b][hp][64:128, :], ah_last)
        queue_outproj(b)
        drain_aux(len(aux_q))

    nc.compile()
    return nc


def _get_nc():
    if "nc" not in _NC_CACHE:
        _NC_CACHE["nc"] = build_nc()
    return _NC_CACHE["nc"]


def run_kernel(x, W_qkv, b_qkv, W_out, b_out, trace=False, **trace_kw):
    from concourse.bass_utils import run_bass_kernel_spmd

    nc = _get_nc()
    xs = np.ascontiguousarray(x, dtype=np.float32).reshape(B_FULL, C, S)
    shards = xs.reshape(N_CORES, B_LOC, C, S)
    common = {
        "W_qkv": np.ascontiguousarray(W_qkv, dtype=np.float32),
        "b_qkv": np.ascontiguousarray(b_qkv, dtype=np.float32),
        "W_out": np.ascontiguousarray(W_out, dtype=np.float32),
        "b_out": np.ascontiguousarray(b_out, dtype=np.float32),
    }
    in_maps = [{"x": np.ascontiguousarray(shards[i]), **common} for i in range(N_CORES)]
    res = run_bass_kernel_spmd(
        nc, in_maps, core_ids=list(range(N_CORES)), trace=trace, **trace_kw
    )
    out = np.stack([res.results[i]["out"] for i in range(N_CORES)])
    hw = int(round(np.sqrt(S)))
    return out.reshape(B_FULL, C, hw, hw).astype(np.float32), res


def kernel(x, W_qkv, b_qkv, W_out, b_out):
    out, _ = run_kernel(x, W_qkv, b_qkv, W_out, b_out)
    return out
